# revision 2
# baseline (speedup 1.0000x reference)
"""Masked dot-product attention (B=16, Q=K=2048, D=512, fp32) on 8 TRN2 cores.

Data-parallel over batch: each of the 8 NeuronCores runs full [Q,K] attention
for 2 batches. Per batch, on-chip:

  S-pass :  S[q,k] tiles = Qt.T @ Kt   (f32r matmuls, PSUM [128,512] x4 banks)
            DVE adds additive key mask (-30000 on masked cols -> exp == 0),
            ACT exp -> W-staging tile, accum_out gives row-sum partials free.
  St-pass:  S^T[k,q] tiles = Kt.T @ Qt (same operands, swapped roles) with the
            mask applied as a per-partition ACT bias; exp -> f32r lhsT tiles
            for the context matmul (avoids transposing the 2048^2 weights).
  W      :  in-place ACT Copy with per-partition scale = 1/rowsum, DMA out.
  ctx    :  PSUM accum of expSt.T @ V over 16 k-tiles, DVE scale by 1/rowsum.

Q is prescaled by 1/sqrt(D) on the host; masks are data (program is
value-agnostic w.r.t. valid_lens).
"""

import numpy as np

import concourse.bass as bass  # noqa: F401  (bass types used via bacc/tile)
import concourse.mybir as mybir
import concourse.tile as tile
from concourse import bacc
from concourse.bass_utils import run_bass_kernel_spmd

F32 = mybir.dt.float32
F32R = mybir.dt.float32r
AF = mybir.ActivationFunctionType
ALU = mybir.AluOpType

B, NQ, NK, ND = 16, 2048, 2048, 512
NCORES = 8
BPC = B // NCORES  # batches per core
MASK_NEG = -30000.0


def _make_identity(nc, ident):
    nc.gpsimd.memset(ident, 0.0)
    sq = ident.shape[0]
    nc.gpsimd.affine_select(
        out=ident,
        in_=ident,
        compare_op=ALU.not_equal,
        fill=1.0,
        base=0,
        pattern=[[-1, sq]],
        channel_multiplier=1,
    )


def build_nc(bpc=BPC, nq=NQ, nk=NK, nd=ND):
    nqt = nq // 128   # q tiles
    nkt = nk // 128   # k tiles
    ndc = nd // 128   # d chunks (contraction)
    nkb = nk // 512   # k blocks (S free dim)
    nqb = nq // 512   # q blocks (St free dim)

    nc = bacc.Bacc(None, target_bir_lowering=False, debug=False)
    q_p = nc.declare_dram_parameter("q", [bpc, nq, nd], F32, isOutput=False)
    k_p = nc.declare_dram_parameter("k", [bpc, nk, nd], F32, isOutput=False)
    v_p = nc.declare_dram_parameter("v", [bpc, nk, nd], F32, isOutput=False)
    mrow_p = nc.declare_dram_parameter("mrow", [bpc, nk], F32, isOutput=False)
    mcol_p = nc.declare_dram_parameter("mcol", [bpc, 128, nkt], F32, isOutput=False)
    w_p = nc.declare_dram_parameter("w", [bpc, nq, nk], F32, isOutput=True)
    ctx_p = nc.declare_dram_parameter("ctx", [bpc, nq, nd], F32, isOutput=True)

    with tile.TileContext(nc) as tc:
        with (
            tc.tile_pool(name="const", bufs=1) as constp,
            tc.tile_pool(name="big", bufs=1) as bigp,
            tc.tile_pool(name="work", bufs=2) as workp,
            tc.tile_pool(name="wpool", bufs=3) as wpool,
            tc.tile_pool(name="small", bufs=4) as smallp,
            tc.tile_pool(name="trp", bufs=2, space="PSUM") as trp,
            tc.tile_pool(name="sps", bufs=4, space="PSUM") as spsp,
            tc.tile_pool(name="stps", bufs=1, space="PSUM") as stpsp,
            tc.tile_pool(name="ctxps", bufs=1, space="PSUM") as ctxpsp,
        ):
            ident = constp.tile([128, 128], F32)
            _make_identity(nc, ident)

            for b in range(bpc):
                mrow = workp.tile([128, nk], F32, tag="mrow")
                nc.sync.dma_start(out=mrow, in_=mrow_p[b, :].partition_broadcast(128))
                mcol = workp.tile([128, nkt], F32, tag="mcol")
                nc.sync.dma_start(out=mcol, in_=mcol_p[b])

                # K -> Kt [d, k] (f32r), via PE transpose of natural tiles
                Kt = bigp.tile([128, ndc, nk], F32R, tag="Kt")
                for kt in range(nkt):
                    k_nat = workp.tile([128, nd], F32, tag="k_nat")
                    nc.sync.dma_start(
                        out=k_nat, in_=k_p[b, kt * 128 : (kt + 1) * 128, :]
                    )
                    for dc in range(ndc):
                        tp = trp.tile([128, 128], F32, tag="tr")
                        nc.tensor.transpose(
                            tp, k_nat[:, dc * 128 : (dc + 1) * 128], ident
                        )
                        nc.vector.tensor_copy(
                            Kt[:, dc, kt * 128 : (kt + 1) * 128], tp
                        )

                # V (natural [k, d]) -> f32r
                Vr = bigp.tile([128, nkt, nd], F32R, tag="Vr")
                for kt in range(nkt):
                    v_nat = workp.tile([128, nd], F32, tag="v_nat")
                    nc.sync.dma_start(
                        out=v_nat, in_=v_p[b, kt * 128 : (kt + 1) * 128, :]
                    )
                    nc.vector.tensor_copy(Vr[:, kt, :], v_nat)

                for qb in range(nqb):
                    # Q tiles of this q-block -> Qt [d, 512] (f32r)
                    Qt = workp.tile([128, ndc, 512], F32R, tag="Qt")
                    for qtl in range(4):
                        qt = qb * 4 + qtl
                        q_nat = workp.tile([128, nd], F32, tag="q_nat")
                        nc.sync.dma_start(
                            out=q_nat, in_=q_p[b, qt * 128 : (qt + 1) * 128, :]
                        )
                        for dc in range(ndc):
                            tp = trp.tile([128, 128], F32, tag="tr")
                            nc.tensor.transpose(
                                tp, q_nat[:, dc * 128 : (dc + 1) * 128], ident
                            )
                            nc.vector.tensor_copy(
                                Qt[:, dc, qtl * 128 : (qtl + 1) * 128], tp
                            )

                    # St pass: expSt[k, q] tiles (mask via ACT bias)
                    expSt = bigp.tile([128, nkt, 512], F32R, tag="expSt")
                    for kt in range(nkt):
                        st = stpsp.tile([128, 512], F32, tag="st")
                        for dc in range(ndc):
                            nc.tensor.matmul(
                                st,
                                Kt[:, dc, kt * 128 : (kt + 1) * 128],
                                Qt[:, dc, :],
                                start=(dc == 0),
                                stop=(dc == ndc - 1),
                            )
                        nc.scalar.activation(
                            expSt[:, kt, :], st, AF.Exp, bias=mcol[:, kt : kt + 1]
                        )

                    # S pass + W output + rowsums
                    rcs = []
                    for qtl in range(4):
                        qt = qb * 4 + qtl
                        wout = wpool.tile([128, nk], F32, tag="wout")
                        sums = smallp.tile([128, nkb], F32, tag="sums")
                        sp_tiles = [
                            spsp.tile([128, 512], F32, tag="sps", name=f"sps{i}")
                            for i in range(nkb)
                        ]
                        for dc in range(ndc):
                            for kb in range(nkb):
                                nc.tensor.matmul(
                                    sp_tiles[kb],
                                    Qt[:, dc, qtl * 128 : (qtl + 1) * 128],
                                    Kt[:, dc, kb * 512 : (kb + 1) * 512],
                                    start=(dc == 0),
                                    stop=(dc == ndc - 1),
                                )
                        for kb in range(nkb):
                            nc.vector.tensor_add(
                                sp_tiles[kb],
                                sp_tiles[kb],
                                mrow[:, kb * 512 : (kb + 1) * 512],
                            )
                            nc.scalar.activation(
                                wout[:, kb * 512 : (kb + 1) * 512],
                                sp_tiles[kb],
                                AF.Exp,
                                accum_out=sums[:, kb : kb + 1],
                            )
                        rs = smallp.tile([128, 1], F32, tag="rs")
                        nc.vector.reduce_sum(rs, sums, axis=mybir.AxisListType.X)
                        rc = smallp.tile([128, 1], F32, tag="rc")
                        nc.vector.reciprocal(rc, rs)
                        rcs.append(rc)
                        nc.scalar.activation(wout, wout, AF.Copy, scale=rc)
                        nc.sync.dma_start(
                            out=w_p[b, qt * 128 : (qt + 1) * 128, :], in_=wout
                        )

                    # context pass
                    for qtl in range(4):
                        qt = qb * 4 + qtl
                        cps = ctxpsp.tile([128, nd], F32, tag="cps")
                        for kt in range(nkt):
                            nc.tensor.matmul(
                                cps,
                                expSt[:, kt, qtl * 128 : (qtl + 1) * 128],
                                Vr[:, kt, :],
                                start=(kt == 0),
                                stop=(kt == nkt - 1),
                            )
                        ctx_sb = workp.tile([128, nd], F32, tag="ctx_sb")
                        nc.vector.tensor_scalar_mul(ctx_sb, cps, rcs[qtl])
                        nc.sync.dma_start(
                            out=ctx_p[b, qt * 128 : (qt + 1) * 128, :], in_=ctx_sb
                        )

    nc.finalize()
    return nc


_NC_CACHE = {}


def _get_nc():
    if "nc" not in _NC_CACHE:
        _NC_CACHE["nc"] = build_nc()
    return _NC_CACHE["nc"]


def make_inputs(queries, keys, values, valid_lens):
    """Host-side prep: prescale Q, build additive masks, shard across cores."""
    scale = np.float32(1.0 / np.sqrt(np.float32(queries.shape[-1])))
    q = (np.asarray(queries, dtype=np.float32) * scale).astype(np.float32)
    k = np.ascontiguousarray(np.asarray(keys, dtype=np.float32))
    v = np.ascontiguousarray(np.asarray(values, dtype=np.float32))
    vl = np.asarray(valid_lens).astype(np.int64)
    nk = k.shape[1]
    nkt = nk // 128
    key_pos = np.arange(nk)
    mrow = np.where(key_pos[None, :] < vl[:, None], 0.0, MASK_NEG).astype(np.float32)
    # mcol[b, p, t] = mrow[b, t*128 + p]
    mcol = np.ascontiguousarray(
        mrow.reshape(-1, nkt, 128).transpose(0, 2, 1)
    ).astype(np.float32)

    in_maps = []
    for c in range(NCORES):
        sl = slice(c * BPC, (c + 1) * BPC)
        in_maps.append(
            {
                "q": np.ascontiguousarray(q[sl]),
                "k": np.ascontiguousarray(k[sl]),
                "v": np.ascontiguousarray(v[sl]),
                "mrow": np.ascontiguousarray(mrow[sl]),
                "mcol": np.ascontiguousarray(mcol[sl]),
            }
        )
    return in_maps


def kernel(queries, keys, values, valid_lens):
    in_maps = make_inputs(queries, keys, values, valid_lens)
    nc = _get_nc()
    res = run_bass_kernel_spmd(nc, in_maps, list(range(NCORES)))
    ctx = np.concatenate([res.results[c]["ctx"] for c in range(NCORES)], axis=0)
    w = np.concatenate([res.results[c]["w"] for c in range(NCORES)], axis=0)
    return ctx, w
